# revision 25
# baseline (speedup 1.0000x reference)
"""ContextQueryAttention (BiDAF-style) Trainium2 kernel, 8-core data-parallel.

Math (per batch):
  s[i,j]  = wq.q_j + wc.c_i + sum_d c_id * wcq_d * q_jd          (L1 x L2)
  s1      = softmax_i(s * mq_j + (1-mq_j)*NEG)                   (softmax over i)
  s2      = softmax_i(s * mp_i + (1-mp_i)*NEG)
  a       = s1 @ Q ;  b = s1 @ (s2^T @ C)        (no L1xL1 intermediate)
  out     = [C, a, C*a, C*b]                                      (L1 x 4D)

Key kernel facts:
 - wq.q_j is constant along the softmax axis (i), cancels entirely.
 - cwc_i is FOLDED INTO the e1 score matmul stationary:
     qtw2[d,j] = wcq[d]*q^T[d,j] + wc[d]  =>  sum_d c[i,d]*qtw2[d,j] = dot_ij + cwc_i
   so the e1 EXP reads PSUM directly (scale=mq_j); SHIFT cancels on this path.
 - e2: exp(mp_i*dot + (mp_i*(cwc_i+S) - S)) in one ACT (per-partition
   scale+bias); masked entries underflow to exactly 0.  Z2 via a ones
   column appended to C in the t matmul.
 - PERMUTED ROW ORDER for both sequence axes: row i lives at
   (tile g*4+r, partition p) with i = 512g+4p+r (j likewise, one group).
   4 consecutive DRAM rows land in one partition => 2KB DMA descriptors,
   4x fewer than the naive (t p) layout.  Softmax reductions are
   order-invariant; masks are loaded and outputs written with the same
   permutation, so results are exact.
 - DMA descriptors of one call auto-spread over all 16 queues; per-call
   issue costs ~0.6us+1.2ns/desc on the issuing sequencer, so inputs are
   4+3 calls per batch and each output tile is ONE 128x2KB-desc call.
 - batches are PHASE-SHIFTED: batch0 runs dma->escore->t->ab while
   batch1's dma/escore overlaps batch0's ab, spreading scalar EXP work
   and output DMA bus time over the whole span.
 - transposes read f32 tiles directly (2cyc/row); DVE cast-copies to bf16.
"""

import itertools

import numpy as np

import concourse.bass as bass
import concourse.mybir as mybir
import concourse.tile as tile
from concourse import bacc
from concourse import bass_utils
from concourse.masks import make_identity

F32 = mybir.dt.float32
BF16 = mybir.dt.bfloat16
EXP = mybir.ActivationFunctionType.Exp
IDENT = mybir.ActivationFunctionType.Identity
ADD = mybir.AluOpType.add
MULT = mybir.AluOpType.mult

B, L1, L2, D = 16, 2048, 512, 128
NCORES = 8
BPC = B // NCORES          # batches per core
NT1 = L1 // 128            # 16 i-tiles
NT2 = L2 // 128            # 4  j-tiles
SHIFT = 1000.0             # makes masked E2 entries underflow exp to 0.0


def _build_program(dbg=False):
    nc = bacc.Bacc("TRN2", target_bir_lowering=False, debug=False)

    ctx_d = nc.dram_tensor("context", [BPC, L1, D], F32, kind="ExternalInput").ap()
    qry_d = nc.dram_tensor("query", [BPC, L2, D], F32, kind="ExternalInput").ap()
    w_d = nc.dram_tensor("w", [3, D], F32, kind="ExternalInput").ap()
    mp_d = nc.dram_tensor("mask_p", [BPC, L1], F32, kind="ExternalInput").ap()
    mq_d = nc.dram_tensor("mask_q", [BPC, L2], F32, kind="ExternalInput").ap()
    out_d = nc.dram_tensor("out", [BPC, L1, 4 * D], F32, kind="ExternalOutput").ap()

    with tile.TileContext(nc) as tc:
        with (
            tc.tile_pool(name="const", bufs=1) as const,
            tc.tile_pool(name="big", bufs=2) as big,
            tc.tile_pool(name="work", bufs=2) as work,
            tc.tile_pool(name="outp", bufs=8) as outp,
            tc.tile_pool(name="ps_e1", bufs=2, space="PSUM") as ps_e1,    # 2 banks each
            tc.tile_pool(name="ps_e2", bufs=2, space="PSUM") as ps_e2,    # 1 bank each
            tc.tile_pool(name="ps_misc", bufs=2, space="PSUM") as ps_misc,  # 1 bank each
        ):
            ident_f = const.tile([128, 128], F32)
            make_identity(nc, ident_f)
            w3 = const.tile([3, 128], F32)
            nc.scalar.dma_start(out=w3, in_=w_d)
            psw = ps_misc.tile([128, 4], F32, tag="m", name="psw")
            nc.tensor.transpose(psw[:, 0:3], w3, ident_f[0:3, 0:3])
            w_sb = const.tile([128, 3], F32)  # cols: wq, wc, wcq
            nc.vector.tensor_copy(w_sb, psw[:, 0:3])
            w_b = const.tile([128, 3], BF16)
            nc.vector.tensor_copy(w_b, psw[:, 0:3])

            S = [dict() for _ in range(BPC)]  # per-batch tile state

            def ph_dma(b, eng_a, eng_b):
                """Input DMAs. Each dma_start call lands on ONE queue at
                ~22.5GB/s, so calls are split into ~128KB partition-range
                chunks and issued from two sequencers in parallel."""
                s = S[b]
                # query first (small, unblocks the q-side pipeline early)
                s["qn"] = work.tile([128, NT2 * 128], F32, tag="qn", name=f"qn{b}")
                qn_r = qry_d[b].rearrange("(p r) d -> p (r d)", p=128)
                for h in range(4):
                    eng_a.dma_start(
                        out=s["qn"][32 * h : 32 * (h + 1), :],
                        in_=qn_r[32 * h : 32 * (h + 1), :],
                    )
                # context, permuted: i = 512g + 4p + r -> c4[p, g, 128r+d]
                s["c1"] = big.tile([128, NT1 // 4, 512], F32, tag="c1", name=f"c1_{b}")
                ctx_r = ctx_d[b].rearrange("(g p r) d -> p g (r d)", p=128, r=4)
                for g in range(NT1 // 4):
                    eng = eng_a if g < 2 else eng_b
                    for h in range(4):
                        eng.dma_start(
                            out=s["c1"][32 * h : 32 * (h + 1), g, :],
                            in_=ctx_r[32 * h : 32 * (h + 1), g, :],
                        )
                # masks with matching permutations; column t = tile index
                mp3 = work.tile([128, NT1 // 4, 4], F32, tag="mp", name=f"mp{b}")
                eng_b.dma_start(
                    out=mp3, in_=mp_d[b].rearrange("(g p r) -> p g r", p=128, r=4)
                )
                s["mp"] = mp3.rearrange("p g r -> p (g r)")
                s["mq"] = work.tile([128, NT2], F32, tag="mq", name=f"mq{b}")
                eng_b.dma_start(
                    out=s["mq"], in_=mq_d[b].rearrange("(p r) -> p r", p=128)
                )

            def c1t(s, t):
                """[128,128] f32 slice of permuted context for tile t."""
                return s["c1"][:, t // 4, 128 * (t % 4) : 128 * (t % 4) + 128]

            def ph_qside(b):
                s = S[b]
                qt = work.tile([128, NT2, 128], BF16, tag="qt")
                ps = ps_e2.tile([128, 4, 128], F32, tag="e2")
                for jt in range(NT2):
                    nc.tensor.transpose(
                        ps[:, jt, :], s["qn"][:, 128 * jt : 128 * (jt + 1)], ident_f
                    )
                nc.vector.tensor_copy(qt, ps)
                s["qt"] = qt
                # qtw = wcq*q^T (e2 moving), qtw2 = wcq*q^T + wc (e1 stationary)
                # (single ACTs with per-partition scale/bias; scalar is idle here)
                qtw = work.tile([128, NT2, 128], BF16, tag="qtw")
                nc.gpsimd.tensor_scalar_mul(qtw, qt, w_sb[:, 2:3])
                s["qtw"] = qtw
                qtw2 = work.tile([128, NT2, 128], BF16, tag="qtw2")
                nc.gpsimd.tensor_scalar_add(qtw2, qtw, w_sb[:, 1:2])
                s["qtw2"] = qtw2
                s["e2n"] = big.tile([128, NT1, L2], BF16, tag="e2n", name=f"e2n{b}")
                s["e1"] = big.tile([128, NT2, L1], BF16, tag="e1", name=f"e1_{b}")
                s["z1h"] = work.tile([128, NT2, 2], F32, tag="z1h", name=f"z1h{b}")
                s["ct"] = big.tile([128, NT1, 128], BF16, tag="ct", name=f"ct{b}")
                s["bias2"] = work.tile([128, NT1], F32, tag="b2", name=f"b2_{b}")

            def ph_cq(b, n):
                """c-side quarter: transpose 4 c tiles, cwc+bias2 for them."""
                s = S[b]
                ps = ps_e2.tile([128, 4, 128], F32, tag="e2")
                for k in range(4):
                    nc.tensor.transpose(ps[:, k, :], c1t(s, 4 * n + k), ident_f)
                nc.vector.tensor_copy(s["ct"][:, 4 * n : 4 * (n + 1), :], ps)
                psc = ps_misc.tile([128, 4], F32, tag="m", name=f"psc{b}_{n}")
                for k in range(4):
                    it = 4 * n + k
                    nc.tensor.matmul(
                        psc[:, k : k + 1], s["ct"][:, it, :], w_b[:, 1:2],
                        start=True, stop=True,
                    )
                # bias2 = (cwc + SHIFT)*mp - SHIFT, two non-in-place steps
                # (an in-place DVE tensor_scalar op here hit a ~11us HW stall)
                b2p = work.tile([128, 4], F32, tag="b2p", name=f"b2p{b}_{n}")
                nc.vector.scalar_tensor_tensor(
                    out=b2p, in0=psc, scalar=SHIFT,
                    in1=s["mp"][:, 4 * n : 4 * (n + 1)], op0=ADD, op1=MULT,
                )
                nc.gpsimd.tensor_scalar_add(
                    s["bias2"][:, 4 * n : 4 * (n + 1)], b2p, -SHIFT
                )

            def ph_e2(b, it0, it1):
                s = S[b]
                for it in range(it0, it1):
                    pss = ps_e2.tile([128, 512], F32, tag="e2")
                    nc.tensor.matmul(pss, s["ct"][:, it, :], s["qtw"], start=True, stop=True)
                    nc.scalar.activation(
                        s["e2n"][:, it, :], pss, EXP,
                        bias=s["bias2"][:, it : it + 1],
                        scale=s["mp"][:, it : it + 1],
                    )

            def ph_e1(b, h):
                """e1 half h: columns 1024h..1024h+1024 (ct quarters 2h, 2h+1)."""
                s = S[b]
                for jt in range(NT2):
                    ps1 = ps_e1.tile([128, 1024], F32, tag="e1")
                    for k in range(2):
                        n = 2 * h + k
                        nc.tensor.matmul(
                            ps1[:, 512 * k : 512 * (k + 1)],
                            s["qtw2"][:, jt, :],
                            s["ct"][:, 4 * n : 4 * (n + 1), :],
                            start=True, stop=True,
                        )
                    nc.scalar.activation(
                        s["e1"][:, jt, 1024 * h : 1024 * (h + 1)], ps1, EXP,
                        scale=s["mq"][:, jt : jt + 1],
                        accum_out=s["z1h"][:, jt, h : h + 1],
                    )

            def ph_c1b(b):
                """bf16 [C|1] for the t matmul moving operand; off critical path."""
                s = S[b]
                c1b = big.tile([128, NT1, 129], BF16, tag="c1b", name=f"c1b{b}")
                for it in range(NT1):
                    nc.gpsimd.tensor_copy(c1b[:, it, 0:128], c1t(s, it))
                nc.gpsimd.memset(c1b[:, :, 128:129], 1.0)
                s["c1b"] = c1b

            def ph_t_mm(b, jt0, jt1):
                """t = s2n^T @ [C|1] matmuls; needs only e2n+c1b, so it can
                run while scalar is still draining the e1 ACTs."""
                s = S[b]
                if "tn" not in s:
                    s["tn"] = work.tile([128, NT2, 129], F32, tag="tn", name=f"tn{b}")
                for jt in range(jt0, jt1):
                    pst = ps_misc.tile([128, 129], F32, tag="m")
                    for it in range(NT1):
                        nc.tensor.matmul(
                            pst, s["e2n"][:, it, jt * 128 : (jt + 1) * 128],
                            s["c1b"][:, it, :],
                            start=(it == 0), stop=(it == NT1 - 1),
                        )
                    nc.vector.tensor_copy(s["tn"][:, jt, :], pst)

            def ph_t(b):
                s = S[b]
                z1 = work.tile([128, NT2], F32, tag="z1")
                for jt in range(NT2):
                    nc.vector.tensor_add(
                        z1[:, jt : jt + 1], s["z1h"][:, jt, 0:1], s["z1h"][:, jt, 1:2]
                    )
                rz1 = work.tile([128, NT2], F32, tag="rz1")
                nc.vector.reciprocal(rz1, z1)
                rhs_ab = work.tile([128, NT2, 256], BF16, tag="rhs_ab")
                for jt in range(NT2):
                    rz2 = work.tile([128, 1], F32, tag="rz2")
                    nc.vector.reciprocal(rz2, s["tn"][:, jt, 128:129])
                    rz12 = work.tile([128, 1], F32, tag="rz12")
                    nc.vector.tensor_mul(rz12, rz2, rz1[:, jt : jt + 1])
                    nc.vector.tensor_scalar_mul(
                        rhs_ab[:, jt, 128:256], s["tn"][:, jt, 0:128], rz12
                    )
                    nc.vector.tensor_scalar_mul(
                        rhs_ab[:, jt, 0:128],
                        s["qn"][:, 128 * jt : 128 * (jt + 1)],
                        rz1[:, jt : jt + 1],
                    )
                s["rhs_ab"] = rhs_ab

            def ph_ab(b, it, eng):
                s = S[b]
                psab = ps_misc.tile([128, 256], F32, tag="m")
                for jt in range(NT2):
                    nc.tensor.matmul(
                        psab,
                        s["e1"][:, jt, it * 128 : (it + 1) * 128],
                        s["rhs_ab"][:, jt, :],
                        start=(jt == 0), stop=(jt == NT2 - 1),
                    )
                o_sb = outp.tile([128, 512], F32, tag="o_sb")
                ceng = nc.vector if b == BPC - 1 else nc.gpsimd
                ceng.tensor_copy(o_sb[:, 0:128], c1t(s, it))
                nc.vector.tensor_copy(o_sb[:, 128:256], psab[:, 0:128])
                nc.vector.tensor_mul(o_sb[:, 256:384], c1t(s, it), psab[:, 0:128])
                nc.vector.tensor_mul(o_sb[:, 384:512], c1t(s, it), psab[:, 128:256])
                # un-permute: partition p -> DRAM row 512g+4p+r, one 2KB desc per p.
                # each call lands on ONE queue (~5.7us per 128KB), so split
                # tiles into halves, and the final tiles into quarters.
                out_r = out_d[b].rearrange("(g p r) c -> p g r c", p=128, r=4)
                nh = 4 if (b == BPC - 1 and it >= NT1 - 8) else 2
                rows = 128 // nh
                if b == BPC - 1:
                    rot = [nc.sync, nc.scalar, nc.gpsimd]
                    engs = [rot[(it + hh) % 3] for hh in range(nh)]
                else:
                    engs = [eng] * nh
                for hh in range(nh):
                    engs[hh].dma_start(
                        out=out_r[rows * hh : rows * (hh + 1), it // 4, it % 4, :],
                        in_=o_sb[rows * hh : rows * (hh + 1), :],
                    )

            def ph_dbg(b):
                if not (dbg and b == 0):
                    return
                s = S[b]
                for name, key in [
                    ("dbg_e1", "e1"), ("dbg_e2n", "e2n"),
                    ("dbg_bias2", "bias2"), ("dbg_rhs_ab", "rhs_ab"),
                    ("dbg_ct", "ct"), ("dbg_qt", "qt"), ("dbg_qtw2", "qtw2"),
                ]:
                    src = s[key]
                    dd = nc.dram_tensor(
                        name, list(src.shape), src.dtype, kind="ExternalOutput"
                    ).ap()
                    nc.sync.dma_start(out=dd, in_=src)

            # ---- emission: phase-shifted batch pipeline --------------------
            ph_dma(0, nc.sync, nc.gpsimd)
            ph_qside(0)
            ph_cq(0, 0); ph_e2(0, 0, 4)
            ph_cq(0, 1); ph_e2(0, 4, 8)
            ph_cq(0, 2); ph_e2(0, 8, 12)
            ph_cq(0, 3); ph_e2(0, 12, 16)
            ph_c1b(0)
            ph_dma(1, nc.sync, nc.gpsimd)
            ph_e1(0, 0); ph_e1(0, 1)
            # PE-independent fill while scalar drains e1(0) ACTs:
            ph_qside(1)
            ph_t_mm(0, 0, 2)
            ph_cq(1, 0); ph_cq(1, 1)
            ph_t_mm(0, 2, 4)
            ph_cq(1, 2); ph_cq(1, 3)
            ph_t(0)
            ph_ab(0, 0, nc.sync); ph_ab(0, 1, nc.sync)
            ph_e2(1, 0, 4)
            ph_ab(0, 2, nc.sync); ph_ab(0, 3, nc.sync)
            ph_e2(1, 4, 8)
            ph_ab(0, 4, nc.sync); ph_ab(0, 5, nc.sync)
            ph_e2(1, 8, 12)
            ph_ab(0, 6, nc.sync); ph_ab(0, 7, nc.sync)
            ph_e2(1, 12, 16)
            ph_ab(0, 8, nc.sync); ph_ab(0, 9, nc.sync)
            ph_c1b(1)
            ph_e1(1, 0)
            ph_ab(0, 10, nc.sync); ph_ab(0, 11, nc.sync); ph_ab(0, 12, nc.sync)
            ph_e1(1, 1)
            ph_ab(0, 13, nc.sync); ph_ab(0, 14, nc.sync)
            ph_t_mm(1, 0, 2)
            ph_ab(0, 15, nc.sync)
            ph_t_mm(1, 2, 4)
            ph_t(1)
            for it in range(NT1):
                ph_ab(1, it, nc.sync)
            ph_dbg(0)

    nc.compile()
    return nc


_NC = None


def _get_nc():
    global _NC
    if _NC is None:
        _NC = _build_program()
    return _NC


def _make_in_maps(inputs):
    context, query, w = inputs["context"], inputs["query"], inputs["w"]
    w2 = np.ascontiguousarray(np.asarray(w).reshape(3, D).astype(np.float32))
    mp = np.asarray(inputs["mask_p"]).astype(np.float32)
    mq = np.asarray(inputs["mask_q"]).astype(np.float32)
    in_maps = []
    for c in range(NCORES):
        sl = slice(c * BPC, (c + 1) * BPC)
        in_maps.append(
            {
                "context": np.ascontiguousarray(context[sl]),
                "query": np.ascontiguousarray(query[sl]),
                "w": w2,
                "mask_p": np.ascontiguousarray(mp[sl]),
                "mask_q": np.ascontiguousarray(mq[sl]),
            }
        )
    return in_maps


def kernel(context, query, w, mask_p, mask_q):
    nc = _get_nc()
    in_maps = _make_in_maps(
        {"context": context, "query": query, "w": w, "mask_p": mask_p, "mask_q": mask_q}
    )
    res = bass_utils.run_bass_kernel_spmd(nc, in_maps, core_ids=list(range(NCORES)))
    return np.concatenate([res.results[c]["out"] for c in range(NCORES)], axis=0)


# revision 26
# speedup vs baseline: 1.4196x; 1.4196x over previous
"""ContextQueryAttention (BiDAF-style) Trainium2 kernel, 8-core data-parallel.

Math (per batch):
  s[i,j]  = wq.q_j + wc.c_i + sum_d c_id * wcq_d * q_jd          (L1 x L2)
  s1      = softmax_i(s * mq_j + (1-mq_j)*NEG)                   (softmax over i)
  s2      = softmax_i(s * mp_i + (1-mp_i)*NEG)
  a       = s1 @ Q ;  b = s1 @ (s2^T @ C)        (no L1xL1 intermediate)
  out     = [C, a, C*a, C*b]                                      (L1 x 4D)

Key kernel facts:
 - wq.q_j is constant along the softmax axis (i), cancels entirely.
 - cwc_i is FOLDED INTO the e1 score matmul stationary:
     qtw2[d,j] = wcq[d]*q^T[d,j] + wc[d]  =>  sum_d c[i,d]*qtw2[d,j] = dot_ij + cwc_i
   so the e1 EXP reads PSUM directly (scale=mq_j); SHIFT cancels on this path.
 - e2: exp(mp_i*dot + (mp_i*(cwc_i+S) - S)) in one ACT (per-partition
   scale+bias); masked entries underflow to exactly 0.  Z2 via a ones
   column appended to C in the t matmul.
 - PERMUTED ROW ORDER for both sequence axes: row i lives at
   (tile g*4+r, partition p) with i = 512g+4p+r (j likewise, one group).
   4 consecutive DRAM rows land in one partition => 2KB DMA descriptors,
   4x fewer than the naive (t p) layout.  Softmax reductions are
   order-invariant; masks are loaded and outputs written with the same
   permutation, so results are exact.
 - DMA descriptors of one call auto-spread over all 16 queues; per-call
   issue costs ~0.6us+1.2ns/desc on the issuing sequencer, so inputs are
   4+3 calls per batch and each output tile is ONE 128x2KB-desc call.
 - batches are PHASE-SHIFTED: batch0 runs dma->escore->t->ab while
   batch1's dma/escore overlaps batch0's ab, spreading scalar EXP work
   and output DMA bus time over the whole span.
 - transposes read f32 tiles directly (2cyc/row); DVE cast-copies to bf16.
"""

import itertools

import numpy as np

import concourse.bass as bass
import concourse.mybir as mybir
import concourse.tile as tile
from concourse import bacc
from concourse import bass_utils
from concourse.masks import make_identity

F32 = mybir.dt.float32
BF16 = mybir.dt.bfloat16
EXP = mybir.ActivationFunctionType.Exp
IDENT = mybir.ActivationFunctionType.Identity
ADD = mybir.AluOpType.add
MULT = mybir.AluOpType.mult

B, L1, L2, D = 16, 2048, 512, 128
NCORES = 8
BPC = B // NCORES          # batches per core
NT1 = L1 // 128            # 16 i-tiles
NT2 = L2 // 128            # 4  j-tiles
SHIFT = 1000.0             # makes masked E2 entries underflow exp to 0.0


def _build_program(dbg=False):
    nc = bacc.Bacc("TRN2", target_bir_lowering=False, debug=False)

    ctx_d = nc.dram_tensor("context", [BPC, L1, D], F32, kind="ExternalInput").ap()
    qry_d = nc.dram_tensor("query", [BPC, L2, D], F32, kind="ExternalInput").ap()
    w_d = nc.dram_tensor("w", [3, D], F32, kind="ExternalInput").ap()
    mp_d = nc.dram_tensor("mask_p", [BPC, L1], F32, kind="ExternalInput").ap()
    mq_d = nc.dram_tensor("mask_q", [BPC, L2], F32, kind="ExternalInput").ap()
    out_d = nc.dram_tensor("out", [BPC, L1, 4 * D], F32, kind="ExternalOutput").ap()

    with tile.TileContext(nc) as tc:
        with (
            tc.tile_pool(name="const", bufs=1) as const,
            tc.tile_pool(name="big", bufs=2) as big,
            tc.tile_pool(name="work", bufs=2) as work,
            tc.tile_pool(name="outp", bufs=8) as outp,
            tc.tile_pool(name="ps_e1", bufs=2, space="PSUM") as ps_e1,    # 2 banks each
            tc.tile_pool(name="ps_e2", bufs=2, space="PSUM") as ps_e2,    # 1 bank each
            tc.tile_pool(name="ps_misc", bufs=2, space="PSUM") as ps_misc,  # 1 bank each
        ):
            ident_f = const.tile([128, 128], F32)
            make_identity(nc, ident_f)
            w3 = const.tile([3, 128], F32)
            nc.scalar.dma_start(out=w3, in_=w_d)
            psw = ps_misc.tile([128, 4], F32, tag="m", name="psw")
            nc.tensor.transpose(psw[:, 0:3], w3, ident_f[0:3, 0:3])
            w_sb = const.tile([128, 3], F32)  # cols: wq, wc, wcq
            nc.vector.tensor_copy(w_sb, psw[:, 0:3])
            w_b = const.tile([128, 3], BF16)
            nc.vector.tensor_copy(w_b, psw[:, 0:3])

            S = [dict() for _ in range(BPC)]  # per-batch tile state

            def ph_dma(b, eng_a, eng_b):
                """Input DMAs. Each dma_start call lands on ONE queue at
                ~22.5GB/s, so calls are split into ~128KB partition-range
                chunks and issued from two sequencers in parallel."""
                s = S[b]
                # query first (small, unblocks the q-side pipeline early)
                s["qn"] = work.tile([128, NT2 * 128], F32, tag="qn", name=f"qn{b}")
                qn_r = qry_d[b].rearrange("(p r) d -> p (r d)", p=128)
                for h in range(4):
                    eng_a.dma_start(
                        out=s["qn"][32 * h : 32 * (h + 1), :],
                        in_=qn_r[32 * h : 32 * (h + 1), :],
                    )
                # context, permuted: i = 512g + 4p + r -> c4[p, g, 128r+d]
                s["c1"] = big.tile([128, NT1 // 4, 512], F32, tag="c1", name=f"c1_{b}")
                ctx_r = ctx_d[b].rearrange("(g p r) d -> p g (r d)", p=128, r=4)
                for g in range(NT1 // 4):
                    eng = eng_a if g < 2 else eng_b
                    for h in range(4):
                        eng.dma_start(
                            out=s["c1"][32 * h : 32 * (h + 1), g, :],
                            in_=ctx_r[32 * h : 32 * (h + 1), g, :],
                        )
                # masks with matching permutations; column t = tile index
                mp3 = work.tile([128, NT1 // 4, 4], F32, tag="mp", name=f"mp{b}")
                eng_b.dma_start(
                    out=mp3, in_=mp_d[b].rearrange("(g p r) -> p g r", p=128, r=4)
                )
                s["mp"] = mp3.rearrange("p g r -> p (g r)")
                s["mq"] = work.tile([128, NT2], F32, tag="mq", name=f"mq{b}")
                eng_b.dma_start(
                    out=s["mq"], in_=mq_d[b].rearrange("(p r) -> p r", p=128)
                )

            def c1t(s, t):
                """[128,128] f32 slice of permuted context for tile t."""
                return s["c1"][:, t // 4, 128 * (t % 4) : 128 * (t % 4) + 128]

            def ph_qside(b):
                s = S[b]
                qt = work.tile([128, NT2, 128], BF16, tag="qt")
                ps = ps_e2.tile([128, 4, 128], F32, tag="e2")
                for jt in range(NT2):
                    nc.tensor.transpose(
                        ps[:, jt, :], s["qn"][:, 128 * jt : 128 * (jt + 1)], ident_f
                    )
                nc.vector.tensor_copy(qt, ps)
                s["qt"] = qt
                # qtw = wcq*q^T (e2 moving), qtw2 = wcq*q^T + wc (e1 stationary)
                # (single ACTs with per-partition scale/bias; scalar is idle here)
                qtw = work.tile([128, NT2, 128], BF16, tag="qtw")
                nc.scalar.activation(qtw, qt, IDENT, scale=w_sb[:, 2:3])
                s["qtw"] = qtw
                qtw2 = work.tile([128, NT2, 128], BF16, tag="qtw2")
                nc.scalar.activation(
                    qtw2, qt, IDENT, scale=w_sb[:, 2:3], bias=w_sb[:, 1:2]
                )
                s["qtw2"] = qtw2
                s["e2n"] = big.tile([128, NT1, L2], BF16, tag="e2n", name=f"e2n{b}")
                s["e1"] = big.tile([128, NT2, L1], BF16, tag="e1", name=f"e1_{b}")
                s["z1h"] = work.tile([128, NT2, 2], F32, tag="z1h", name=f"z1h{b}")
                s["ct"] = big.tile([128, NT1, 128], BF16, tag="ct", name=f"ct{b}")
                s["bias2"] = work.tile([128, NT1], F32, tag="b2", name=f"b2_{b}")

            def ph_cq(b, n):
                """c-side quarter: transpose 4 c tiles, cwc+bias2 for them."""
                s = S[b]
                ps = ps_e2.tile([128, 4, 128], F32, tag="e2")
                for k in range(4):
                    nc.tensor.transpose(ps[:, k, :], c1t(s, 4 * n + k), ident_f)
                nc.vector.tensor_copy(s["ct"][:, 4 * n : 4 * (n + 1), :], ps)
                psc = ps_misc.tile([128, 4], F32, tag="m", name=f"psc{b}_{n}")
                for k in range(4):
                    it = 4 * n + k
                    nc.tensor.matmul(
                        psc[:, k : k + 1], s["ct"][:, it, :], w_b[:, 1:2],
                        start=True, stop=True,
                    )
                # bias2 = (cwc + SHIFT)*mp - SHIFT, two non-in-place steps
                # (an in-place DVE tensor_scalar op here hit a ~11us HW stall)
                b2p = work.tile([128, 4], F32, tag="b2p", name=f"b2p{b}_{n}")
                nc.vector.scalar_tensor_tensor(
                    out=b2p, in0=psc, scalar=SHIFT,
                    in1=s["mp"][:, 4 * n : 4 * (n + 1)], op0=ADD, op1=MULT,
                )
                nc.gpsimd.tensor_scalar_add(
                    s["bias2"][:, 4 * n : 4 * (n + 1)], b2p, -SHIFT
                )

            def ph_e2(b, it0, it1):
                s = S[b]
                for it in range(it0, it1):
                    pss = ps_e2.tile([128, 512], F32, tag="e2")
                    nc.tensor.matmul(pss, s["ct"][:, it, :], s["qtw"], start=True, stop=True)
                    nc.scalar.activation(
                        s["e2n"][:, it, :], pss, EXP,
                        bias=s["bias2"][:, it : it + 1],
                        scale=s["mp"][:, it : it + 1],
                    )

            def ph_e1(b, h):
                """e1 half h: columns 1024h..1024h+1024 (ct quarters 2h, 2h+1)."""
                s = S[b]
                for jt in range(NT2):
                    ps1 = ps_e1.tile([128, 1024], F32, tag="e1")
                    for k in range(2):
                        n = 2 * h + k
                        nc.tensor.matmul(
                            ps1[:, 512 * k : 512 * (k + 1)],
                            s["qtw2"][:, jt, :],
                            s["ct"][:, 4 * n : 4 * (n + 1), :],
                            start=True, stop=True,
                        )
                    nc.scalar.activation(
                        s["e1"][:, jt, 1024 * h : 1024 * (h + 1)], ps1, EXP,
                        scale=s["mq"][:, jt : jt + 1],
                        accum_out=s["z1h"][:, jt, h : h + 1],
                    )

            def ph_c1b(b):
                """bf16 [C|1] for the t matmul moving operand; off critical path."""
                s = S[b]
                c1b = big.tile([128, NT1, 129], BF16, tag="c1b", name=f"c1b{b}")
                for it in range(NT1):
                    nc.gpsimd.tensor_copy(c1b[:, it, 0:128], c1t(s, it))
                nc.gpsimd.memset(c1b[:, :, 128:129], 1.0)
                s["c1b"] = c1b

            def ph_t_mm(b, jt0, jt1):
                """t = s2n^T @ [C|1] matmuls; needs only e2n+c1b, so it can
                run while scalar is still draining the e1 ACTs."""
                s = S[b]
                if "tn" not in s:
                    s["tn"] = work.tile([128, NT2, 129], F32, tag="tn", name=f"tn{b}")
                for jt in range(jt0, jt1):
                    pst = ps_misc.tile([128, 129], F32, tag="m")
                    for it in range(NT1):
                        nc.tensor.matmul(
                            pst, s["e2n"][:, it, jt * 128 : (jt + 1) * 128],
                            s["c1b"][:, it, :],
                            start=(it == 0), stop=(it == NT1 - 1),
                        )
                    nc.vector.tensor_copy(s["tn"][:, jt, :], pst)

            def ph_t(b):
                s = S[b]
                z1 = work.tile([128, NT2], F32, tag="z1")
                for jt in range(NT2):
                    nc.vector.tensor_add(
                        z1[:, jt : jt + 1], s["z1h"][:, jt, 0:1], s["z1h"][:, jt, 1:2]
                    )
                rz1 = work.tile([128, NT2], F32, tag="rz1")
                nc.vector.reciprocal(rz1, z1)
                rhs_ab = work.tile([128, NT2, 256], BF16, tag="rhs_ab")
                for jt in range(NT2):
                    rz2 = work.tile([128, 1], F32, tag="rz2")
                    nc.vector.reciprocal(rz2, s["tn"][:, jt, 128:129])
                    rz12 = work.tile([128, 1], F32, tag="rz12")
                    nc.vector.tensor_mul(rz12, rz2, rz1[:, jt : jt + 1])
                    nc.vector.tensor_scalar_mul(
                        rhs_ab[:, jt, 128:256], s["tn"][:, jt, 0:128], rz12
                    )
                    nc.vector.tensor_scalar_mul(
                        rhs_ab[:, jt, 0:128],
                        s["qn"][:, 128 * jt : 128 * (jt + 1)],
                        rz1[:, jt : jt + 1],
                    )
                s["rhs_ab"] = rhs_ab

            def ph_ab(b, it, eng):
                s = S[b]
                psab = ps_misc.tile([128, 256], F32, tag="m")
                for jt in range(NT2):
                    nc.tensor.matmul(
                        psab,
                        s["e1"][:, jt, it * 128 : (it + 1) * 128],
                        s["rhs_ab"][:, jt, :],
                        start=(jt == 0), stop=(jt == NT2 - 1),
                    )
                o_sb = outp.tile([128, 512], F32, tag="o_sb")
                ceng = nc.vector if b == BPC - 1 else nc.gpsimd
                ceng.tensor_copy(o_sb[:, 0:128], c1t(s, it))
                nc.vector.tensor_copy(o_sb[:, 128:256], psab[:, 0:128])
                nc.vector.tensor_mul(o_sb[:, 256:384], c1t(s, it), psab[:, 0:128])
                nc.vector.tensor_mul(o_sb[:, 384:512], c1t(s, it), psab[:, 128:256])
                # un-permute: partition p -> DRAM row 512g+4p+r, one 2KB desc per p.
                # each call lands on ONE queue (~5.7us per 128KB), so split
                # tiles into halves, and the final tiles into quarters.
                out_r = out_d[b].rearrange("(g p r) c -> p g r c", p=128, r=4)
                nh = 4 if (b == BPC - 1 and it >= NT1 - 8) else 2
                rows = 128 // nh
                if b == BPC - 1:
                    rot = [nc.sync, nc.scalar, nc.gpsimd]
                    engs = [rot[(it + hh) % 3] for hh in range(nh)]
                else:
                    engs = [eng] * nh
                for hh in range(nh):
                    engs[hh].dma_start(
                        out=out_r[rows * hh : rows * (hh + 1), it // 4, it % 4, :],
                        in_=o_sb[rows * hh : rows * (hh + 1), :],
                    )

            def ph_dbg(b):
                if not (dbg and b == 0):
                    return
                s = S[b]
                for name, key in [
                    ("dbg_e1", "e1"), ("dbg_e2n", "e2n"),
                    ("dbg_bias2", "bias2"), ("dbg_rhs_ab", "rhs_ab"),
                    ("dbg_ct", "ct"), ("dbg_qt", "qt"), ("dbg_qtw2", "qtw2"),
                ]:
                    src = s[key]
                    dd = nc.dram_tensor(
                        name, list(src.shape), src.dtype, kind="ExternalOutput"
                    ).ap()
                    nc.sync.dma_start(out=dd, in_=src)

            # ---- emission: phase-shifted batch pipeline --------------------
            ph_dma(0, nc.sync, nc.gpsimd)
            ph_qside(0)
            ph_cq(0, 0); ph_e2(0, 0, 4)
            ph_cq(0, 1); ph_e2(0, 4, 8)
            ph_cq(0, 2); ph_e2(0, 8, 12)
            ph_cq(0, 3); ph_e2(0, 12, 16)
            ph_c1b(0)
            ph_dma(1, nc.sync, nc.gpsimd)
            ph_e1(0, 0); ph_e1(0, 1)
            # PE-independent fill while scalar drains e1(0) ACTs:
            ph_qside(1)
            ph_t_mm(0, 0, 2)
            ph_cq(1, 0); ph_cq(1, 1)
            ph_t_mm(0, 2, 4)
            ph_cq(1, 2); ph_cq(1, 3)
            ph_t(0)
            ph_ab(0, 0, nc.sync); ph_ab(0, 1, nc.sync)
            ph_e2(1, 0, 4)
            ph_ab(0, 2, nc.sync); ph_ab(0, 3, nc.sync)
            ph_e2(1, 4, 8)
            ph_ab(0, 4, nc.sync); ph_ab(0, 5, nc.sync)
            ph_e2(1, 8, 12)
            ph_ab(0, 6, nc.sync); ph_ab(0, 7, nc.sync)
            ph_e2(1, 12, 16)
            ph_ab(0, 8, nc.sync); ph_ab(0, 9, nc.sync)
            ph_c1b(1)
            ph_e1(1, 0)
            ph_ab(0, 10, nc.sync); ph_ab(0, 11, nc.sync); ph_ab(0, 12, nc.sync)
            ph_e1(1, 1)
            ph_ab(0, 13, nc.sync); ph_ab(0, 14, nc.sync)
            ph_t_mm(1, 0, 2)
            ph_ab(0, 15, nc.sync)
            ph_t_mm(1, 2, 4)
            ph_t(1)
            for it in range(NT1):
                ph_ab(1, it, nc.sync)
            ph_dbg(0)

    nc.compile()
    return nc


_NC = None


def _get_nc():
    global _NC
    if _NC is None:
        _NC = _build_program()
    return _NC


def _make_in_maps(inputs):
    context, query, w = inputs["context"], inputs["query"], inputs["w"]
    w2 = np.ascontiguousarray(np.asarray(w).reshape(3, D).astype(np.float32))
    mp = np.asarray(inputs["mask_p"]).astype(np.float32)
    mq = np.asarray(inputs["mask_q"]).astype(np.float32)
    in_maps = []
    for c in range(NCORES):
        sl = slice(c * BPC, (c + 1) * BPC)
        in_maps.append(
            {
                "context": np.ascontiguousarray(context[sl]),
                "query": np.ascontiguousarray(query[sl]),
                "w": w2,
                "mask_p": np.ascontiguousarray(mp[sl]),
                "mask_q": np.ascontiguousarray(mq[sl]),
            }
        )
    return in_maps


def kernel(context, query, w, mask_p, mask_q):
    nc = _get_nc()
    in_maps = _make_in_maps(
        {"context": context, "query": query, "w": w, "mask_p": mask_p, "mask_q": mask_q}
    )
    res = bass_utils.run_bass_kernel_spmd(nc, in_maps, core_ids=list(range(NCORES)))
    return np.concatenate([res.results[c]["out"] for c in range(NCORES)], axis=0)


# revision 27
# speedup vs baseline: 1.5809x; 1.1136x over previous
"""ContextQueryAttention (BiDAF-style) Trainium2 kernel, 8-core data-parallel.

Math (per batch):
  s[i,j]  = wq.q_j + wc.c_i + sum_d c_id * wcq_d * q_jd          (L1 x L2)
  s1      = softmax_i(s * mq_j + (1-mq_j)*NEG)                   (softmax over i)
  s2      = softmax_i(s * mp_i + (1-mp_i)*NEG)
  a       = s1 @ Q ;  b = s1 @ (s2^T @ C)        (no L1xL1 intermediate)
  out     = [C, a, C*a, C*b]                                      (L1 x 4D)

Key kernel facts:
 - wq.q_j is constant along the softmax axis (i), cancels entirely.
 - cwc_i is FOLDED INTO the e1 score matmul stationary:
     qtw2[d,j] = wcq[d]*q^T[d,j] + wc[d]  =>  sum_d c[i,d]*qtw2[d,j] = dot_ij + cwc_i
   so the e1 EXP reads PSUM directly (scale=mq_j); SHIFT cancels on this path.
 - e2: exp(mp_i*dot + (mp_i*(cwc_i+S) - S)) in one ACT (per-partition
   scale+bias); masked entries underflow to exactly 0.  Z2 via a ones
   column appended to C in the t matmul.
 - PERMUTED ROW ORDER for both sequence axes: row i lives at
   (tile g*4+r, partition p) with i = 512g+4p+r (j likewise, one group).
   4 consecutive DRAM rows land in one partition => 2KB DMA descriptors,
   4x fewer than the naive (t p) layout.  Softmax reductions are
   order-invariant; masks are loaded and outputs written with the same
   permutation, so results are exact.
 - DMA descriptors of one call auto-spread over all 16 queues; per-call
   issue costs ~0.6us+1.2ns/desc on the issuing sequencer, so inputs are
   4+3 calls per batch and each output tile is ONE 128x2KB-desc call.
 - batches are PHASE-SHIFTED: batch0 runs dma->escore->t->ab while
   batch1's dma/escore overlaps batch0's ab, spreading scalar EXP work
   and output DMA bus time over the whole span.
 - transposes read f32 tiles directly (2cyc/row); DVE cast-copies to bf16.
"""

import itertools

import numpy as np

import concourse.bass as bass
import concourse.mybir as mybir
import concourse.tile as tile
from concourse import bacc
from concourse import bass_utils
from concourse.masks import make_identity

F32 = mybir.dt.float32
BF16 = mybir.dt.bfloat16
EXP = mybir.ActivationFunctionType.Exp
IDENT = mybir.ActivationFunctionType.Identity
ADD = mybir.AluOpType.add
MULT = mybir.AluOpType.mult

B, L1, L2, D = 16, 2048, 512, 128
NCORES = 8
BPC = B // NCORES          # batches per core
NT1 = L1 // 128            # 16 i-tiles
NT2 = L2 // 128            # 4  j-tiles
SHIFT = 1000.0             # makes masked E2 entries underflow exp to 0.0


def _build_program(dbg=False):
    nc = bacc.Bacc("TRN2", target_bir_lowering=False, debug=False)

    ctx_d = nc.dram_tensor("context", [BPC, L1, D], F32, kind="ExternalInput").ap()
    qry_d = nc.dram_tensor("query", [BPC, L2, D], F32, kind="ExternalInput").ap()
    w_d = nc.dram_tensor("w", [3, D], F32, kind="ExternalInput").ap()
    mp_d = nc.dram_tensor("mask_p", [BPC, L1], F32, kind="ExternalInput").ap()
    mq_d = nc.dram_tensor("mask_q", [BPC, L2], F32, kind="ExternalInput").ap()
    out_d = nc.dram_tensor("out", [BPC, L1, 4 * D], F32, kind="ExternalOutput").ap()

    with tile.TileContext(nc) as tc:
        with (
            tc.tile_pool(name="const", bufs=1) as const,
            tc.tile_pool(name="big", bufs=2) as big,
            tc.tile_pool(name="work", bufs=2) as work,
            tc.tile_pool(name="outp", bufs=8) as outp,
            tc.tile_pool(name="ps_e1", bufs=2, space="PSUM") as ps_e1,    # 2 banks each
            tc.tile_pool(name="ps_e2", bufs=2, space="PSUM") as ps_e2,    # 1 bank each
            tc.tile_pool(name="ps_misc", bufs=2, space="PSUM") as ps_misc,  # 1 bank each
        ):
            ident_f = const.tile([128, 128], F32)
            make_identity(nc, ident_f)
            w3 = const.tile([3, 128], F32)
            nc.scalar.dma_start(out=w3, in_=w_d)
            psw = ps_misc.tile([128, 4], F32, tag="m", name="psw")
            nc.tensor.transpose(psw[:, 0:3], w3, ident_f[0:3, 0:3])
            w_sb = const.tile([128, 3], F32)  # cols: wq, wc, wcq
            nc.vector.tensor_copy(w_sb, psw[:, 0:3])
            w_b = const.tile([128, 3], BF16)
            nc.vector.tensor_copy(w_b, psw[:, 0:3])

            S = [dict() for _ in range(BPC)]  # per-batch tile state

            def ph_dma(b, eng_a, eng_b):
                """Input DMAs. Each dma_start call lands on ONE queue at
                ~22.5GB/s, so calls are split into ~128KB partition-range
                chunks and issued from two sequencers in parallel."""
                s = S[b]
                # query first (small, unblocks the q-side pipeline early)
                s["qn"] = work.tile([128, NT2 * 128], F32, tag="qn", name=f"qn{b}")
                qn_r = qry_d[b].rearrange("(p r) d -> p (r d)", p=128)
                for h in range(4):
                    eng_a.dma_start(
                        out=s["qn"][32 * h : 32 * (h + 1), :],
                        in_=qn_r[32 * h : 32 * (h + 1), :],
                    )
                # context, permuted: i = 512g + 4p + r -> c4[p, g, 128r+d]
                s["c1"] = big.tile([128, NT1 // 4, 512], F32, tag="c1", name=f"c1_{b}")
                ctx_r = ctx_d[b].rearrange("(g p r) d -> p g (r d)", p=128, r=4)
                for g in range(NT1 // 4):
                    eng = eng_a if g < 2 else eng_b
                    for h in range(4):
                        eng.dma_start(
                            out=s["c1"][32 * h : 32 * (h + 1), g, :],
                            in_=ctx_r[32 * h : 32 * (h + 1), g, :],
                        )
                # masks with matching permutations; column t = tile index
                mp3 = work.tile([128, NT1 // 4, 4], F32, tag="mp", name=f"mp{b}")
                eng_b.dma_start(
                    out=mp3, in_=mp_d[b].rearrange("(g p r) -> p g r", p=128, r=4)
                )
                s["mp"] = mp3.rearrange("p g r -> p (g r)")
                s["mq"] = work.tile([128, NT2], F32, tag="mq", name=f"mq{b}")
                eng_b.dma_start(
                    out=s["mq"], in_=mq_d[b].rearrange("(p r) -> p r", p=128)
                )

            def c1t(s, t):
                """[128,128] f32 slice of permuted context for tile t."""
                return s["c1"][:, t // 4, 128 * (t % 4) : 128 * (t % 4) + 128]

            def ph_qside(b):
                s = S[b]
                qt = work.tile([128, NT2, 128], BF16, tag="qt")
                ps = ps_e2.tile([128, 4, 128], F32, tag="e2")
                for jt in range(NT2):
                    nc.tensor.transpose(
                        ps[:, jt, :], s["qn"][:, 128 * jt : 128 * (jt + 1)], ident_f
                    )
                nc.vector.tensor_copy(qt, ps)
                s["qt"] = qt
                # qtw = wcq*q^T (e2 moving), qtw2 = wcq*q^T + wc (e1 stationary)
                # (single ACTs with per-partition scale/bias; scalar is idle here)
                qtw = work.tile([128, NT2, 128], BF16, tag="qtw")
                nc.scalar.activation(qtw, qt, IDENT, scale=w_sb[:, 2:3])
                s["qtw"] = qtw
                qtw2 = work.tile([128, NT2, 128], BF16, tag="qtw2")
                nc.scalar.activation(
                    qtw2, qt, IDENT, scale=w_sb[:, 2:3], bias=w_sb[:, 1:2]
                )
                s["qtw2"] = qtw2
                s["e2n"] = big.tile([128, NT1, L2], BF16, tag="e2n", name=f"e2n{b}")
                s["e1"] = big.tile([128, NT2, L1], BF16, tag="e1", name=f"e1_{b}")
                s["z1h"] = work.tile([128, NT2, 2], F32, tag="z1h", name=f"z1h{b}")
                s["ct"] = big.tile([128, NT1, 128], BF16, tag="ct", name=f"ct{b}")
                s["bias2"] = work.tile([128, NT1], F32, tag="b2", name=f"b2_{b}")

            def ph_cq(b, n):
                """c-side quarter: transpose 4 c tiles, cwc+bias2 for them."""
                s = S[b]
                ps = ps_e2.tile([128, 4, 128], F32, tag="e2")
                for k in range(4):
                    nc.tensor.transpose(ps[:, k, :], c1t(s, 4 * n + k), ident_f)
                nc.vector.tensor_copy(s["ct"][:, 4 * n : 4 * (n + 1), :], ps)
                psc = ps_misc.tile([128, 4], F32, tag="m", name=f"psc{b}_{n}")
                for k in range(4):
                    it = 4 * n + k
                    nc.tensor.matmul(
                        psc[:, k : k + 1], s["ct"][:, it, :], w_b[:, 1:2],
                        start=True, stop=True,
                    )
                # bias2 = (cwc + SHIFT)*mp - SHIFT, two non-in-place steps
                # (an in-place DVE tensor_scalar op here hit a ~11us HW stall)
                b2p = work.tile([128, 4], F32, tag="b2p", name=f"b2p{b}_{n}")
                nc.vector.scalar_tensor_tensor(
                    out=b2p, in0=psc, scalar=SHIFT,
                    in1=s["mp"][:, 4 * n : 4 * (n + 1)], op0=ADD, op1=MULT,
                )
                nc.gpsimd.tensor_scalar_add(
                    s["bias2"][:, 4 * n : 4 * (n + 1)], b2p, -SHIFT
                )

            def ph_e2(b, it0, it1):
                s = S[b]
                for it in range(it0, it1):
                    pss = ps_e2.tile([128, 512], F32, tag="e2")
                    nc.tensor.matmul(pss, s["ct"][:, it, :], s["qtw"], start=True, stop=True)
                    nc.scalar.activation(
                        s["e2n"][:, it, :], pss, EXP,
                        bias=s["bias2"][:, it : it + 1],
                        scale=s["mp"][:, it : it + 1],
                    )

            def ph_e1(b, h):
                """e1 half h: columns 1024h..1024h+1024 (ct quarters 2h, 2h+1)."""
                s = S[b]
                for jt in range(NT2):
                    ps1 = ps_e1.tile([128, 1024], F32, tag="e1")
                    for k in range(2):
                        n = 2 * h + k
                        nc.tensor.matmul(
                            ps1[:, 512 * k : 512 * (k + 1)],
                            s["qtw2"][:, jt, :],
                            s["ct"][:, 4 * n : 4 * (n + 1), :],
                            start=True, stop=True,
                        )
                    nc.scalar.activation(
                        s["e1"][:, jt, 1024 * h : 1024 * (h + 1)], ps1, EXP,
                        scale=s["mq"][:, jt : jt + 1],
                        accum_out=s["z1h"][:, jt, h : h + 1],
                    )

            def ph_c1b(b):
                """bf16 [C|1] for the t matmul moving operand; off critical path."""
                s = S[b]
                c1b = big.tile([128, NT1, 129], BF16, tag="c1b", name=f"c1b{b}")
                for it in range(NT1):
                    nc.gpsimd.tensor_copy(c1b[:, it, 0:128], c1t(s, it))
                nc.gpsimd.memset(c1b[:, :, 128:129], 1.0)
                s["c1b"] = c1b

            def ph_t_mm(b, jt0, jt1):
                """t = s2n^T @ [C|1] matmuls; needs only e2n+c1b, so it can
                run while scalar is still draining the e1 ACTs."""
                s = S[b]
                if "tn" not in s:
                    s["tn"] = work.tile([128, NT2, 129], F32, tag="tn", name=f"tn{b}")
                for jt in range(jt0, jt1):
                    pst = ps_misc.tile([128, 129], F32, tag="m")
                    for it in range(NT1):
                        nc.tensor.matmul(
                            pst, s["e2n"][:, it, jt * 128 : (jt + 1) * 128],
                            s["c1b"][:, it, :],
                            start=(it == 0), stop=(it == NT1 - 1),
                        )
                    nc.vector.tensor_copy(s["tn"][:, jt, :], pst)

            def ph_t(b):
                s = S[b]
                z1 = work.tile([128, NT2], F32, tag="z1")
                for jt in range(NT2):
                    nc.vector.tensor_add(
                        z1[:, jt : jt + 1], s["z1h"][:, jt, 0:1], s["z1h"][:, jt, 1:2]
                    )
                rz1 = work.tile([128, NT2], F32, tag="rz1")
                nc.vector.reciprocal(rz1, z1)
                rhs_ab = work.tile([128, NT2, 256], BF16, tag="rhs_ab")
                for jt in range(NT2):
                    rz2 = work.tile([128, 1], F32, tag="rz2")
                    nc.vector.reciprocal(rz2, s["tn"][:, jt, 128:129])
                    rz12 = work.tile([128, 1], F32, tag="rz12")
                    nc.vector.tensor_mul(rz12, rz2, rz1[:, jt : jt + 1])
                    nc.vector.tensor_scalar_mul(
                        rhs_ab[:, jt, 128:256], s["tn"][:, jt, 0:128], rz12
                    )
                    nc.vector.tensor_scalar_mul(
                        rhs_ab[:, jt, 0:128],
                        s["qn"][:, 128 * jt : 128 * (jt + 1)],
                        rz1[:, jt : jt + 1],
                    )
                s["rhs_ab"] = rhs_ab

            def ph_ab(b, it, eng):
                s = S[b]
                psab = ps_misc.tile([128, 256], F32, tag="m")
                for jt in range(NT2):
                    nc.tensor.matmul(
                        psab,
                        s["e1"][:, jt, it * 128 : (it + 1) * 128],
                        s["rhs_ab"][:, jt, :],
                        start=(jt == 0), stop=(jt == NT2 - 1),
                    )
                o_sb = outp.tile([128, 512], F32, tag="o_sb")
                ceng = nc.vector if b == BPC - 1 else nc.gpsimd
                ceng.tensor_copy(o_sb[:, 0:128], c1t(s, it))
                nc.vector.tensor_copy(o_sb[:, 128:256], psab[:, 0:128])
                nc.vector.tensor_mul(o_sb[:, 256:384], c1t(s, it), psab[:, 0:128])
                nc.vector.tensor_mul(o_sb[:, 384:512], c1t(s, it), psab[:, 128:256])
                # un-permute: partition p -> DRAM row 512g+4p+r, one 2KB desc per p.
                # each call lands on ONE queue (~5.7us per 128KB), so split
                # tiles into halves, and the final tiles into quarters.
                out_r = out_d[b].rearrange("(g p r) c -> p g r c", p=128, r=4)
                nh = 4 if (b == BPC - 1 and it >= NT1 - 8) else 2
                rows = 128 // nh
                if b == BPC - 1:
                    rot = [nc.sync, nc.scalar, nc.gpsimd]
                    engs = [rot[(it + hh) % 3] for hh in range(nh)]
                else:
                    engs = [eng] * nh
                for hh in range(nh):
                    engs[hh].dma_start(
                        out=out_r[rows * hh : rows * (hh + 1), it // 4, it % 4, :],
                        in_=o_sb[rows * hh : rows * (hh + 1), :],
                    )

            def ph_dbg(b):
                if not (dbg and b == 0):
                    return
                s = S[b]
                for name, key in [
                    ("dbg_e1", "e1"), ("dbg_e2n", "e2n"),
                    ("dbg_bias2", "bias2"), ("dbg_rhs_ab", "rhs_ab"),
                    ("dbg_ct", "ct"), ("dbg_qt", "qt"), ("dbg_qtw2", "qtw2"),
                ]:
                    src = s[key]
                    dd = nc.dram_tensor(
                        name, list(src.shape), src.dtype, kind="ExternalOutput"
                    ).ap()
                    nc.sync.dma_start(out=dd, in_=src)

            # ---- emission: phase-shifted batch pipeline --------------------
            ph_dma(0, nc.sync, nc.gpsimd)
            ph_qside(0)
            ph_cq(0, 0); ph_cq(0, 1); ph_cq(0, 2); ph_cq(0, 3)
            ph_e2(0, 0, 16)        # all e2(0) ACTs ahead of e1(0) on scalar
            ph_c1b(0)
            ph_dma(1, nc.sync, nc.gpsimd)
            ph_e1(0, 0); ph_e1(0, 1)
            # PE-independent fill while scalar drains e1(0) ACTs:
            ph_qside(1)
            ph_t_mm(0, 0, 2)
            ph_cq(1, 0); ph_cq(1, 1)
            ph_t_mm(0, 2, 4)
            ph_cq(1, 2); ph_cq(1, 3)
            ph_t(0)
            ph_ab(0, 0, nc.sync); ph_ab(0, 1, nc.sync)
            ph_e2(1, 0, 4)
            ph_ab(0, 2, nc.sync); ph_ab(0, 3, nc.sync)
            ph_e2(1, 4, 8)
            ph_ab(0, 4, nc.sync); ph_ab(0, 5, nc.sync)
            ph_e2(1, 8, 12)
            ph_ab(0, 6, nc.sync); ph_ab(0, 7, nc.sync)
            ph_e2(1, 12, 16)
            ph_ab(0, 8, nc.sync); ph_ab(0, 9, nc.sync)
            ph_c1b(1)
            ph_e1(1, 0)
            ph_ab(0, 10, nc.sync); ph_ab(0, 11, nc.sync); ph_ab(0, 12, nc.sync)
            ph_e1(1, 1)
            ph_ab(0, 13, nc.sync); ph_ab(0, 14, nc.sync)
            ph_t_mm(1, 0, 2)
            ph_ab(0, 15, nc.sync)
            ph_t_mm(1, 2, 4)
            ph_t(1)
            for it in range(NT1):
                ph_ab(1, it, nc.sync)
            ph_dbg(0)

    nc.compile()
    return nc


_NC = None


def _get_nc():
    global _NC
    if _NC is None:
        _NC = _build_program()
    return _NC


def _make_in_maps(inputs):
    context, query, w = inputs["context"], inputs["query"], inputs["w"]
    w2 = np.ascontiguousarray(np.asarray(w).reshape(3, D).astype(np.float32))
    mp = np.asarray(inputs["mask_p"]).astype(np.float32)
    mq = np.asarray(inputs["mask_q"]).astype(np.float32)
    in_maps = []
    for c in range(NCORES):
        sl = slice(c * BPC, (c + 1) * BPC)
        in_maps.append(
            {
                "context": np.ascontiguousarray(context[sl]),
                "query": np.ascontiguousarray(query[sl]),
                "w": w2,
                "mask_p": np.ascontiguousarray(mp[sl]),
                "mask_q": np.ascontiguousarray(mq[sl]),
            }
        )
    return in_maps


def kernel(context, query, w, mask_p, mask_q):
    nc = _get_nc()
    in_maps = _make_in_maps(
        {"context": context, "query": query, "w": w, "mask_p": mask_p, "mask_q": mask_q}
    )
    res = bass_utils.run_bass_kernel_spmd(nc, in_maps, core_ids=list(range(NCORES)))
    return np.concatenate([res.results[c]["out"] for c in range(NCORES)], axis=0)


# revision 28
# speedup vs baseline: 1.6436x; 1.0396x over previous
"""ContextQueryAttention (BiDAF-style) Trainium2 kernel, 8-core data-parallel.

Math (per batch):
  s[i,j]  = wq.q_j + wc.c_i + sum_d c_id * wcq_d * q_jd          (L1 x L2)
  s1      = softmax_i(s * mq_j + (1-mq_j)*NEG)                   (softmax over i)
  s2      = softmax_i(s * mp_i + (1-mp_i)*NEG)
  a       = s1 @ Q ;  b = s1 @ (s2^T @ C)        (no L1xL1 intermediate)
  out     = [C, a, C*a, C*b]                                      (L1 x 4D)

Key kernel facts:
 - wq.q_j is constant along the softmax axis (i), cancels entirely.
 - cwc_i is FOLDED INTO the e1 score matmul stationary:
     qtw2[d,j] = wcq[d]*q^T[d,j] + wc[d]  =>  sum_d c[i,d]*qtw2[d,j] = dot_ij + cwc_i
   so the e1 EXP reads PSUM directly (scale=mq_j); SHIFT cancels on this path.
 - e2: exp(mp_i*dot + (mp_i*(cwc_i+S) - S)) in one ACT (per-partition
   scale+bias); masked entries underflow to exactly 0.  Z2 via a ones
   column appended to C in the t matmul.
 - PERMUTED ROW ORDER for both sequence axes: row i lives at
   (tile g*4+r, partition p) with i = 512g+4p+r (j likewise, one group).
   4 consecutive DRAM rows land in one partition => 2KB DMA descriptors,
   4x fewer than the naive (t p) layout.  Softmax reductions are
   order-invariant; masks are loaded and outputs written with the same
   permutation, so results are exact.
 - DMA descriptors of one call auto-spread over all 16 queues; per-call
   issue costs ~0.6us+1.2ns/desc on the issuing sequencer, so inputs are
   4+3 calls per batch and each output tile is ONE 128x2KB-desc call.
 - batches are PHASE-SHIFTED: batch0 runs dma->escore->t->ab while
   batch1's dma/escore overlaps batch0's ab, spreading scalar EXP work
   and output DMA bus time over the whole span.
 - transposes read f32 tiles directly (2cyc/row); DVE cast-copies to bf16.
"""

import itertools

import numpy as np

import concourse.bass as bass
import concourse.mybir as mybir
import concourse.tile as tile
from concourse import bacc
from concourse import bass_utils
from concourse.masks import make_identity

F32 = mybir.dt.float32
BF16 = mybir.dt.bfloat16
EXP = mybir.ActivationFunctionType.Exp
IDENT = mybir.ActivationFunctionType.Identity
ADD = mybir.AluOpType.add
MULT = mybir.AluOpType.mult

B, L1, L2, D = 16, 2048, 512, 128
NCORES = 8
BPC = B // NCORES          # batches per core
NT1 = L1 // 128            # 16 i-tiles
NT2 = L2 // 128            # 4  j-tiles
SHIFT = 1000.0             # makes masked E2 entries underflow exp to 0.0


def _build_program(dbg=False):
    nc = bacc.Bacc("TRN2", target_bir_lowering=False, debug=False)

    ctx_d = nc.dram_tensor("context", [BPC, L1, D], F32, kind="ExternalInput").ap()
    qry_d = nc.dram_tensor("query", [BPC, L2, D], F32, kind="ExternalInput").ap()
    w_d = nc.dram_tensor("w", [3, D], F32, kind="ExternalInput").ap()
    mp_d = nc.dram_tensor("mask_p", [BPC, L1], F32, kind="ExternalInput").ap()
    mq_d = nc.dram_tensor("mask_q", [BPC, L2], F32, kind="ExternalInput").ap()
    out_d = nc.dram_tensor("out", [BPC, L1, 4 * D], F32, kind="ExternalOutput").ap()

    with tile.TileContext(nc) as tc:
        with (
            tc.tile_pool(name="const", bufs=1) as const,
            tc.tile_pool(name="big", bufs=2) as big,
            tc.tile_pool(name="work", bufs=2) as work,
            tc.tile_pool(name="outp", bufs=8) as outp,
            tc.tile_pool(name="ps_e1", bufs=2, space="PSUM") as ps_e1,    # 2 banks each
            tc.tile_pool(name="ps_e2", bufs=2, space="PSUM") as ps_e2,    # 1 bank each
            tc.tile_pool(name="ps_misc", bufs=2, space="PSUM") as ps_misc,  # 1 bank each
        ):
            ident_f = const.tile([128, 128], F32)
            make_identity(nc, ident_f)
            w3 = const.tile([3, 128], F32)
            nc.scalar.dma_start(out=w3, in_=w_d)
            psw = ps_misc.tile([128, 4], F32, tag="m", name="psw")
            nc.tensor.transpose(psw[:, 0:3], w3, ident_f[0:3, 0:3])
            w_sb = const.tile([128, 3], F32)  # cols: wq, wc, wcq
            nc.vector.tensor_copy(w_sb, psw[:, 0:3])
            w_b = const.tile([128, 3], BF16)
            nc.vector.tensor_copy(w_b, psw[:, 0:3])

            S = [dict() for _ in range(BPC)]  # per-batch tile state

            def ph_dma(b, eng_a, eng_b):
                """Input DMAs. Each dma_start call lands on ONE queue at
                ~22.5GB/s, so calls are split into ~128KB partition-range
                chunks and issued from two sequencers in parallel."""
                s = S[b]
                # query first (small, unblocks the q-side pipeline early)
                s["qn"] = work.tile([128, NT2 * 128], F32, tag="qn", name=f"qn{b}")
                qn_r = qry_d[b].rearrange("(p r) d -> p (r d)", p=128)
                for h in range(4):
                    eng_a.dma_start(
                        out=s["qn"][32 * h : 32 * (h + 1), :],
                        in_=qn_r[32 * h : 32 * (h + 1), :],
                    )
                # context, permuted: i = 512g + 4p + r -> c4[p, g, 128r+d]
                s["c1"] = big.tile([128, NT1 // 4, 512], F32, tag="c1", name=f"c1_{b}")
                ctx_r = ctx_d[b].rearrange("(g p r) d -> p g (r d)", p=128, r=4)
                for g in range(NT1 // 4):
                    eng = eng_a if g < 2 else eng_b
                    for h in range(4):
                        eng.dma_start(
                            out=s["c1"][32 * h : 32 * (h + 1), g, :],
                            in_=ctx_r[32 * h : 32 * (h + 1), g, :],
                        )
                # masks with matching permutations; column t = tile index
                mp3 = work.tile([128, NT1 // 4, 4], F32, tag="mp", name=f"mp{b}")
                eng_b.dma_start(
                    out=mp3, in_=mp_d[b].rearrange("(g p r) -> p g r", p=128, r=4)
                )
                s["mp"] = mp3.rearrange("p g r -> p (g r)")
                s["mq"] = work.tile([128, NT2], F32, tag="mq", name=f"mq{b}")
                eng_b.dma_start(
                    out=s["mq"], in_=mq_d[b].rearrange("(p r) -> p r", p=128)
                )

            def c1t(s, t):
                """[128,128] f32 slice of permuted context for tile t."""
                return s["c1"][:, t // 4, 128 * (t % 4) : 128 * (t % 4) + 128]

            def ph_qside(b):
                s = S[b]
                qt = work.tile([128, NT2, 128], BF16, tag="qt")
                ps = ps_e2.tile([128, 4, 128], F32, tag="e2")
                for jt in range(NT2):
                    nc.tensor.transpose(
                        ps[:, jt, :], s["qn"][:, 128 * jt : 128 * (jt + 1)], ident_f
                    )
                nc.vector.tensor_copy(qt, ps)
                s["qt"] = qt
                # qtw = wcq*q^T (e2 moving), qtw2 = wcq*q^T + wc (e1 stationary)
                # (single ACTs with per-partition scale/bias; scalar is idle here)
                qtw = work.tile([128, NT2, 128], BF16, tag="qtw")
                nc.scalar.activation(qtw, qt, IDENT, scale=w_sb[:, 2:3])
                s["qtw"] = qtw
                qtw2 = work.tile([128, NT2, 128], BF16, tag="qtw2")
                nc.scalar.activation(
                    qtw2, qt, IDENT, scale=w_sb[:, 2:3], bias=w_sb[:, 1:2]
                )
                s["qtw2"] = qtw2
                s["e2n"] = big.tile([128, NT1, L2], BF16, tag="e2n", name=f"e2n{b}")
                s["e1"] = big.tile([128, NT2, L1], BF16, tag="e1", name=f"e1_{b}")
                s["z1h"] = work.tile([128, NT2, 2], F32, tag="z1h", name=f"z1h{b}")
                s["ct"] = big.tile([128, NT1, 128], BF16, tag="ct", name=f"ct{b}")
                s["bias2"] = work.tile([128, NT1], F32, tag="b2", name=f"b2_{b}")

            def ph_cq(b, n):
                """c-side quarter: transpose 4 c tiles, cwc+bias2 for them."""
                s = S[b]
                ps = ps_e2.tile([128, 4, 128], F32, tag="e2")
                for k in range(4):
                    nc.tensor.transpose(ps[:, k, :], c1t(s, 4 * n + k), ident_f)
                nc.vector.tensor_copy(s["ct"][:, 4 * n : 4 * (n + 1), :], ps)
                psc = ps_misc.tile([128, 4], F32, tag="m", name=f"psc{b}_{n}")
                for k in range(4):
                    it = 4 * n + k
                    nc.tensor.matmul(
                        psc[:, k : k + 1], s["ct"][:, it, :], w_b[:, 1:2],
                        start=True, stop=True,
                    )
                # bias2 = (cwc + SHIFT)*mp - SHIFT, two non-in-place steps
                # (an in-place DVE tensor_scalar op here hit a ~11us HW stall)
                b2p = work.tile([128, 4], F32, tag="b2p", name=f"b2p{b}_{n}")
                nc.vector.scalar_tensor_tensor(
                    out=b2p, in0=psc, scalar=SHIFT,
                    in1=s["mp"][:, 4 * n : 4 * (n + 1)], op0=ADD, op1=MULT,
                )
                nc.gpsimd.tensor_scalar_add(
                    s["bias2"][:, 4 * n : 4 * (n + 1)], b2p, -SHIFT
                )

            def ph_e2(b, it0, it1):
                s = S[b]
                for it in range(it0, it1):
                    pss = ps_e2.tile([128, 512], F32, tag="e2")
                    nc.tensor.matmul(pss, s["ct"][:, it, :], s["qtw"], start=True, stop=True)
                    nc.scalar.activation(
                        s["e2n"][:, it, :], pss, EXP,
                        bias=s["bias2"][:, it : it + 1],
                        scale=s["mp"][:, it : it + 1],
                    )

            def ph_e1(b, h):
                """e1 half h: columns 1024h..1024h+1024 (ct quarters 2h, 2h+1)."""
                s = S[b]
                for jt in range(NT2):
                    ps1 = ps_e1.tile([128, 1024], F32, tag="e1")
                    for k in range(2):
                        n = 2 * h + k
                        nc.tensor.matmul(
                            ps1[:, 512 * k : 512 * (k + 1)],
                            s["qtw2"][:, jt, :],
                            s["ct"][:, 4 * n : 4 * (n + 1), :],
                            start=True, stop=True,
                        )
                    nc.scalar.activation(
                        s["e1"][:, jt, 1024 * h : 1024 * (h + 1)], ps1, EXP,
                        scale=s["mq"][:, jt : jt + 1],
                        accum_out=s["z1h"][:, jt, h : h + 1],
                    )

            def ph_c1b(b):
                """bf16 [C|1] for the t matmul moving operand.  b0's casts go
                to vector (idle at that point; pool's stream would wedge them
                behind DMA issues and stall the pst chain)."""
                s = S[b]
                eng = nc.vector if b == 0 else nc.gpsimd
                c1b = big.tile([128, NT1, 129], BF16, tag="c1b", name=f"c1b{b}")
                for it in range(NT1):
                    eng.tensor_copy(c1b[:, it, 0:128], c1t(s, it))
                nc.gpsimd.memset(c1b[:, :, 128:129], 1.0)
                s["c1b"] = c1b

            def ph_t_mm(b, jt0, jt1):
                """t = s2n^T @ [C|1] matmuls; needs only e2n+c1b, so it can
                run while scalar is still draining the e1 ACTs."""
                s = S[b]
                if "tn" not in s:
                    s["tn"] = work.tile([128, NT2, 129], F32, tag="tn", name=f"tn{b}")
                for jt in range(jt0, jt1):
                    pst = ps_misc.tile([128, 129], F32, tag="m")
                    for it in range(NT1):
                        nc.tensor.matmul(
                            pst, s["e2n"][:, it, jt * 128 : (jt + 1) * 128],
                            s["c1b"][:, it, :],
                            start=(it == 0), stop=(it == NT1 - 1),
                        )
                    nc.vector.tensor_copy(s["tn"][:, jt, :], pst)

            def ph_t(b):
                s = S[b]
                z1 = work.tile([128, NT2], F32, tag="z1")
                for jt in range(NT2):
                    nc.vector.tensor_add(
                        z1[:, jt : jt + 1], s["z1h"][:, jt, 0:1], s["z1h"][:, jt, 1:2]
                    )
                rz1 = work.tile([128, NT2], F32, tag="rz1")
                nc.vector.reciprocal(rz1, z1)
                rhs_ab = work.tile([128, NT2, 256], BF16, tag="rhs_ab")
                for jt in range(NT2):
                    rz2 = work.tile([128, 1], F32, tag="rz2")
                    nc.vector.reciprocal(rz2, s["tn"][:, jt, 128:129])
                    rz12 = work.tile([128, 1], F32, tag="rz12")
                    nc.vector.tensor_mul(rz12, rz2, rz1[:, jt : jt + 1])
                    nc.vector.tensor_scalar_mul(
                        rhs_ab[:, jt, 128:256], s["tn"][:, jt, 0:128], rz12
                    )
                    nc.vector.tensor_scalar_mul(
                        rhs_ab[:, jt, 0:128],
                        s["qn"][:, 128 * jt : 128 * (jt + 1)],
                        rz1[:, jt : jt + 1],
                    )
                s["rhs_ab"] = rhs_ab

            def ph_ab(b, it, eng):
                s = S[b]
                psab = ps_misc.tile([128, 256], F32, tag="m")
                for jt in range(NT2):
                    nc.tensor.matmul(
                        psab,
                        s["e1"][:, jt, it * 128 : (it + 1) * 128],
                        s["rhs_ab"][:, jt, :],
                        start=(jt == 0), stop=(jt == NT2 - 1),
                    )
                o_sb = outp.tile([128, 512], F32, tag="o_sb")
                ceng = nc.vector if b == BPC - 1 else nc.gpsimd
                ceng.tensor_copy(o_sb[:, 0:128], c1t(s, it))
                nc.vector.tensor_copy(o_sb[:, 128:256], psab[:, 0:128])
                nc.vector.tensor_mul(o_sb[:, 256:384], c1t(s, it), psab[:, 0:128])
                nc.vector.tensor_mul(o_sb[:, 384:512], c1t(s, it), psab[:, 128:256])
                # un-permute: partition p -> DRAM row 512g+4p+r, one 2KB desc per p.
                # each call lands on ONE queue (~5.7us per 128KB), so split
                # tiles into halves, and the final tiles into quarters.
                out_r = out_d[b].rearrange("(g p r) c -> p g r c", p=128, r=4)
                nh = 4 if b == BPC - 1 else 2
                rows = 128 // nh
                if b == BPC - 1:
                    rot = [nc.sync, nc.scalar, nc.gpsimd]
                    engs = [rot[(it + hh) % 3] for hh in range(nh)]
                else:
                    engs = [eng] * nh
                for hh in range(nh):
                    engs[hh].dma_start(
                        out=out_r[rows * hh : rows * (hh + 1), it // 4, it % 4, :],
                        in_=o_sb[rows * hh : rows * (hh + 1), :],
                    )

            def ph_dbg(b):
                if not (dbg and b == 0):
                    return
                s = S[b]
                for name, key in [
                    ("dbg_e1", "e1"), ("dbg_e2n", "e2n"),
                    ("dbg_bias2", "bias2"), ("dbg_rhs_ab", "rhs_ab"),
                    ("dbg_ct", "ct"), ("dbg_qt", "qt"), ("dbg_qtw2", "qtw2"),
                ]:
                    src = s[key]
                    dd = nc.dram_tensor(
                        name, list(src.shape), src.dtype, kind="ExternalOutput"
                    ).ap()
                    nc.sync.dma_start(out=dd, in_=src)

            # ---- emission: phase-shifted batch pipeline --------------------
            ph_dma(0, nc.sync, nc.gpsimd)
            ph_qside(0)
            ph_cq(0, 0); ph_cq(0, 1); ph_cq(0, 2); ph_cq(0, 3)
            ph_e2(0, 0, 16)        # all e2(0) ACTs ahead of e1(0) on scalar
            ph_c1b(0)
            ph_dma(1, nc.sync, nc.gpsimd)
            ph_e1(0, 0); ph_e1(0, 1)
            # PE-independent fill while scalar drains e1(0) ACTs:
            ph_qside(1)
            ph_t_mm(0, 0, 2)
            ph_cq(1, 0); ph_cq(1, 1)
            ph_t_mm(0, 2, 4)
            ph_cq(1, 2); ph_cq(1, 3)
            ph_t(0)
            ph_ab(0, 0, nc.sync); ph_ab(0, 1, nc.sync)
            ph_e2(1, 0, 4)
            ph_ab(0, 2, nc.sync); ph_ab(0, 3, nc.sync)
            ph_e2(1, 4, 8)
            ph_ab(0, 4, nc.sync); ph_ab(0, 5, nc.sync)
            ph_e2(1, 8, 12)
            ph_ab(0, 6, nc.sync); ph_ab(0, 7, nc.sync)
            ph_e2(1, 12, 16)
            ph_ab(0, 8, nc.sync); ph_ab(0, 9, nc.sync)
            ph_c1b(1)
            ph_e1(1, 0)
            ph_ab(0, 10, nc.sync); ph_ab(0, 11, nc.sync); ph_ab(0, 12, nc.sync)
            ph_e1(1, 1)
            ph_ab(0, 13, nc.sync); ph_ab(0, 14, nc.sync)
            ph_t_mm(1, 0, 2)
            ph_ab(0, 15, nc.sync)
            ph_t_mm(1, 2, 4)
            ph_t(1)
            for it in range(NT1):
                ph_ab(1, it, nc.sync)
            ph_dbg(0)

    nc.compile()
    return nc


_NC = None


def _get_nc():
    global _NC
    if _NC is None:
        _NC = _build_program()
    return _NC


def _make_in_maps(inputs):
    context, query, w = inputs["context"], inputs["query"], inputs["w"]
    w2 = np.ascontiguousarray(np.asarray(w).reshape(3, D).astype(np.float32))
    mp = np.asarray(inputs["mask_p"]).astype(np.float32)
    mq = np.asarray(inputs["mask_q"]).astype(np.float32)
    in_maps = []
    for c in range(NCORES):
        sl = slice(c * BPC, (c + 1) * BPC)
        in_maps.append(
            {
                "context": np.ascontiguousarray(context[sl]),
                "query": np.ascontiguousarray(query[sl]),
                "w": w2,
                "mask_p": np.ascontiguousarray(mp[sl]),
                "mask_q": np.ascontiguousarray(mq[sl]),
            }
        )
    return in_maps


def kernel(context, query, w, mask_p, mask_q):
    nc = _get_nc()
    in_maps = _make_in_maps(
        {"context": context, "query": query, "w": w, "mask_p": mask_p, "mask_q": mask_q}
    )
    res = bass_utils.run_bass_kernel_spmd(nc, in_maps, core_ids=list(range(NCORES)))
    return np.concatenate([res.results[c]["out"] for c in range(NCORES)], axis=0)
